# revision 26
# baseline (speedup 1.0000x reference)
"""Trainium2 Bass kernel for nn_Damping_layer: out = kipf_term - lbda[:, None] * input_term.

Sharding: pure row-parallel over the n_nodes axis across 8 NeuronCores
(12500 rows per core), no cross-core communication. Each core's shard is
host-padded to 12544 rows so it divides into 7 uniform tiles of
[128 partitions x 14 rows/partition].

The kernel is HBM-bandwidth bound, so all DRAM I/O is fp16: the host
casts input_term/kipf_term to fp16 (norm relative error ~3e-4, far
inside the 2e-2 gate) and upcasts the fp16 output back to f32. This
halves DRAM traffic vs f32 (19.3 MB/core vs 38.5 MB/core).

input_term and kipf_term are interleaved on host into one DRAM tensor z
(per tile: 1792 input rows then 1792 kipf rows), so each tile needs a
single 1.75 MiB load. Loads and stores alternate between the two HWDGE
rings (SP and ACT) by tile parity, keeping both rings' byte demand equal
so the SDMA engines' per-queue round-robin matches the traffic mix.

lbda is pre-shuffled on host into the matching [partition, group] layout
so the fused DVE op
    out = (input * (-lbda)) + kipf            (InstTensorScalarPtr)
consumes it as a per-partition scalar, one op per 128-row group. The
first/last tiles are emitted as small sub-chunks so the pipeline ramps
in and drains out quickly.
"""

import numpy as np

N_NODES = 100000
N_FEAT = 256
N_CORES = 8
ROWS_PER_CORE = N_NODES // N_CORES  # 12500

R_PP = 14                       # rows per partition in a tile
TILE_ROWS = 128 * R_PP          # 1792 rows per tile
N_TILES = 7                     # tiles per core
PAD_ROWS = N_TILES * TILE_ROWS  # 12544 rows per core after padding
LB_COLS = N_TILES * R_PP        # 98
N_BUFS = 6

_CACHE = {}


def _build_nc():
    from contextlib import ExitStack

    import concourse.bacc as bacc
    import concourse.mybir as mybir
    import concourse.tile as tile

    FP32 = mybir.dt.float32
    FP16 = mybir.dt.float16
    nc = bacc.Bacc(
        "TRN2", target_bir_lowering=False, debug=False, num_devices=N_CORES
    )
    z = nc.dram_tensor(
        "z", [2 * PAD_ROWS, N_FEAT], FP16, kind="ExternalInput"
    ).ap()
    lb = nc.dram_tensor("lb", [128, LB_COLS], FP32, kind="ExternalInput").ap()
    o = nc.dram_tensor("o", [PAD_ROWS, N_FEAT], FP16, kind="ExternalOutput").ap()

    # z layout (host-built): [t, h, p, j, c] with h=0 input rows, h=1 kipf
    # rows; partition p holds R_PP*512B contiguous DRAM per (t, h).
    zv = z.rearrange(
        "(t h p j) c -> t p h (j c)", t=N_TILES, h=2, p=128, j=R_PP
    )
    ov = o.rearrange("(t p j) c -> t p (j c)", t=N_TILES, p=128, j=R_PP)

    MULT = mybir.AluOpType.mult
    ADD = mybir.AluOpType.add
    KOFF = R_PP * N_FEAT  # kipf half offset within a z tile

    with tile.TileContext(nc) as tc, ExitStack() as ctx:
        const = ctx.enter_context(tc.tile_pool(name="const", bufs=1))
        zpool = ctx.enter_context(tc.tile_pool(name="zp", bufs=N_BUFS + 1))
        opool = ctx.enter_context(tc.tile_pool(name="op", bufs=N_BUFS))

        # lbt rides SWDGE (gpsimd), keeping both HWDGE rings' heads free
        # for the first data loads.
        lbt = const.tile([128, LB_COLS], FP32)
        nc.gpsimd.dma_start(out=lbt[:], in_=lb[:])
        nlb = const.tile([128, LB_COLS], FP32)
        nc.vector.tensor_scalar_mul(nlb[:], lbt[:], -1.0)

        # Work list: first/last tiles in small sub-chunks so the pipeline
        # ramps in and drains out quickly; full tiles in between.
        chunks = [(0, 0, 4), (0, 4, 9), (0, 9, 14)]
        chunks += [(t, 0, R_PP) for t in range(1, N_TILES - 1)]
        chunks += [
            (N_TILES - 1, 0, 4),
            (N_TILES - 1, 4, 7),
            (N_TILES - 1, 7, 10),
            (N_TILES - 1, 10, 12),
            (N_TILES - 1, 12, 14),
        ]

        def ld_ring(i):
            # Prefix: tile 0's three sub-chunks on SP and tile 1 (1.75 MiB)
            # on ACT. The engines round-robin their queues per-DESCRIPTOR,
            # so ACT's big full-tile descriptors win ~7x the bytes and t1
            # lands fast for the DVE steady state, while tile 0's small
            # ramp chunks trickle in on SP just ahead of the DVE — this
            # asymmetry is what makes the ramp work. Plain parity after.
            if i < 4:
                return nc.sync if i < 3 else nc.scalar
            return nc.sync if i % 2 == 0 else nc.scalar

        def st_ring(i):
            return nc.scalar if i % 2 == 0 else nc.sync

        def emit_load(i):
            t, jlo, jhi = chunks[i]
            nj = jhi - jlo
            zt = zpool.tile([128, 2 * R_PP * N_FEAT], FP16, tag="zt")
            eng = ld_ring(i)
            if nj == R_PP:
                # whole tile: one 1.75 MiB load covering both halves
                zt_hv = zt[:].rearrange("p (h f) -> p h f", h=2)
                eng.dma_start(out=zt_hv, in_=zv[t])
            else:
                # both halves in ONE dma_start (strided AP): same
                # descriptors, half the head-of-ring dispatch cost.
                zt_hv = zt[:].rearrange("p (h f) -> p h f", h=2)
                eng.dma_start(
                    out=zt_hv[:, :, jlo * N_FEAT : jhi * N_FEAT],
                    in_=zv[t][:, :, jlo * N_FEAT : jhi * N_FEAT],
                )
            return zt

        def emit_compute_store(i, zt):
            t, jlo, jhi = chunks[i]
            ot = opool.tile([128, R_PP * N_FEAT], FP16, tag="ot")
            for j in range(jlo, jhi):
                s = slice(j * N_FEAT, (j + 1) * N_FEAT)
                sk = slice(KOFF + j * N_FEAT, KOFF + (j + 1) * N_FEAT)
                c = t * R_PP + j
                nc.vector.scalar_tensor_tensor(
                    out=ot[:, s],
                    in0=zt[:, s],
                    scalar=nlb[:, c : c + 1],
                    in1=zt[:, sk],
                    op0=MULT,
                    op1=ADD,
                )
            st_ring(i).dma_start(
                out=ov[t][:, jlo * N_FEAT : jhi * N_FEAT],
                in_=ot[:, jlo * N_FEAT : jhi * N_FEAT],
            )

        # Software-pipelined emission: W chunk-loads run ahead so each
        # HWDGE ring's instruction stream starts with pure loads and no
        # store (gated on DVE) ever head-of-line-blocks the load front.
        # W=5 keeps one extra chunk of descriptor backlog per ring to
        # ride out HBM-contention latency spikes (the noisy-mode gaps);
        # W=6 (8 zpool bufs) measured ~4.5 us slower — DMA-write/DVE SBUF
        # contention sets in beyond ~7 outstanding buffers.
        W = 5
        zts = {}
        for i in range(min(W, len(chunks))):
            zts[i] = emit_load(i)
        for i in range(len(chunks)):
            emit_compute_store(i, zts.pop(i))
            if i + W < len(chunks):
                zts[i + W] = emit_load(i + W)

    nc.compile()
    return nc


def _get_nc():
    if "nc" not in _CACHE:
        _CACHE["nc"] = _build_nc()
    return _CACHE["nc"]


def _shuffle_lbda(lb_core):
    """[PAD_ROWS] -> [128, LB_COLS] with lb[p, t*R_PP+j] = lbda[t*TILE_ROWS + p*R_PP + j]."""
    return np.ascontiguousarray(
        lb_core.reshape(N_TILES, 128, R_PP)
        .transpose(1, 0, 2)
        .reshape(128, LB_COLS)
    )


def _make_in_maps(input_term, kipf_term, lbda):
    input_term = np.asarray(input_term, dtype=np.float32)
    kipf_term = np.asarray(kipf_term, dtype=np.float32)
    lbda = np.asarray(lbda, dtype=np.float32)
    in_maps = []
    for c in range(N_CORES):
        sl = slice(c * ROWS_PER_CORE, (c + 1) * ROWS_PER_CORE)
        xpadded = np.zeros((PAD_ROWS, N_FEAT), np.float16)
        xpadded[:ROWS_PER_CORE] = input_term[sl]
        kpadded = np.zeros((PAD_ROWS, N_FEAT), np.float16)
        kpadded[:ROWS_PER_CORE] = kipf_term[sl]
        # z: per tile, TILE_ROWS input rows then TILE_ROWS kipf rows
        zc = np.empty((N_TILES, 2, TILE_ROWS, N_FEAT), np.float16)
        zc[:, 0] = xpadded.reshape(N_TILES, TILE_ROWS, N_FEAT)
        zc[:, 1] = kpadded.reshape(N_TILES, TILE_ROWS, N_FEAT)
        lpadded = np.zeros((PAD_ROWS,), np.float32)
        lpadded[:ROWS_PER_CORE] = lbda[sl]
        in_maps.append(
            {
                "z": zc.reshape(2 * PAD_ROWS, N_FEAT),
                "lb": _shuffle_lbda(lpadded),
            }
        )
    return in_maps


def kernel(input_term, kipf_term, lbda, spar=None, **_unused):
    from concourse.bass_utils import run_bass_kernel_spmd

    nc = _get_nc()
    in_maps = _make_in_maps(input_term, kipf_term, lbda)
    res = run_bass_kernel_spmd(nc, in_maps, list(range(N_CORES))).results
    return np.concatenate(
        [
            res[c]["o"][:ROWS_PER_CORE].astype(np.float32)
            for c in range(N_CORES)
        ],
        axis=0,
    )
